# revision 16
# baseline (speedup 1.0000x reference)
"""MoE layer (top-2 of 8 experts, selection shared across tokens) on 8 TRN2 cores.

Math (faithful to the reference):
    gates = softmax(x @ W_gate + b_gate)          [N, 8]
    idx0  = top-2 expert indices of token 0       [2]
    s     = per-token top-2 gate VALUES (desc)    [N, 2]
    out   = s0 * (x @ W[A] + b[A]) + s1 * (x @ W[B] + b[B])

Strategy v3 (mixed fp16 / fp8-DoubleRow, slice-major cold phase):
  Rotate the two selected expert matrices into the PCA basis of the per-token
  (s0, s1) score cloud: (WP, WQ) = M @ (WA, WB), (u, v) = (s0, s1) @ M^-1.
  ~96.6% of the output power rides the major term u*(x@WP); the minor term
  v*(x@WQ) carries ~3.4%.  The device computes
      out = u' * (x @ WP) + v' * (x @ WQ)     (epilogue on DVE)
  with the Q matmul entirely in fp8-e4m3 DoubleRow (2 k-chunks per matmul,
  ~1.9x effective PE rate at N=512) and the P matmul split: first KF16=14 of
  16 k-chunks in fp16, last 2 as one DoubleRow pair.  Host-simulated rel-err
  1.47e-2 (gate 2e-2).  Scaling: x*8 and W*512 in both precisions so fp16
  and fp8 products share PSUM at scale 4096; epilogue scalars fold in 1/4096.
  Gating + PCA + top-2 + the rank-2 bias term stay on the host (0.2% FLOPs).

Schedule notes (v3, from NTFF analysis of v2):
  - steady-state group (14 fp16 MM + 9 DR MM) = 5070ns, at the HW floor;
    all recoverable time is at the edges.  The 2-queue DMA fabric sustains
    ~350GB/s but only with >=128KB payload per dma_start (~620ns issue each)
    and has ~5us of startup dead time.
  - fp8 x is DERIVED on-device (DVE cast from the resident fp16 x, chunks
    0..13; only chunks 14..15 come via DMA) - removes 3.5MiB of HBM traffic
    and all fine-grained x8 cold streams.
  - phase 1 is SLICE-major: slices q0,q1 run across all 4 column blocks
    (~81us of PE work) while only ~4.8MiB of cold bytes gate the start;
    W1..W3 and x(q2..q7) stream in with large slack.  Block 0 runs its two
    slices k-OUTER across 4 m-groups / 8 PSUM banks in strict k arrival
    order; cold streams are issued in progressive k-levels, first pieces
    smallest.
  - phase 2 runs blocks [3,2,1,0] (W3 still SBUF-resident from phase 1)
    over slices q2..q7; each block prefetches the NEXT block's W at its
    start, before the epilogue out-DMAs that would head-of-line-block the
    FIFO HWDGE queues.  x-casts for q2..q7 are slotted between phase-1
    blocks (DVE is strict FIFO - a cast waiting on DMA there would block
    epilogues and stall PSUM recycling).
  - warm-up matmuls on a DVE-zeroed tile (GpSimd memset cost 2.5us of PE
    wait in v2) hold the PE busy through the DMA head so the HAM clock-gate
    reaches 2.4GHz before the first real matmul.
  - output DMA rides the two HWDGE queues; the final group runs as two
    column-halves to shorten the kernel tail.
"""

import functools

import numpy as np

import concourse.bass as bass
import concourse.mybir as mybir
import concourse.tile as tile
from concourse import bacc
from concourse.bass_utils import run_bass_kernel_spmd

N_CORES = 8
N, D_IN, D_HID = 16384, 2048, 2048
NT = N // N_CORES            # tokens per core
KP = 128                     # contraction chunk = partition dim
KCH = D_IN // KP             # 16 K-chunks
KF16 = 14                    # fp16 k-chunks of the P (major) term
KJ = KCH // 2                # 8 k-pairs (DoubleRow granularity)
NB = 512                     # output column block (1 PSUM bank of fp32)
NBLK = D_HID // NB           # 4 output blocks
TQ = 256                     # token slice per x-stream piece
NQ = NT // TQ                # 8 slices
MPQ = TQ // 128              # m-tiles per slice
NWARM = 8                    # HAM warm-up matmuls (framework boot already
                             # covers ~7.5us of the DMA head)
COLD16 = (2, 2, 4, 6)        # progressive cold k-levels for x16 / wp16

SX = 8.0                     # x quantization scale
SW = 512.0                   # W quantization scale

F32 = mybir.dt.float32
FP16 = mybir.dt.float16
FP8 = mybir.dt.float8e4
DR = mybir.MatmulPerfMode.DoubleRow

O_DT = FP16

# Filled by test harness inspection: last BassKernelResults from a run.
LAST_RESULT = None


@functools.lru_cache(maxsize=1)
def _build():
    nc = bacc.Bacc("TRN2", target_bir_lowering=False, debug=False)
    xT16 = nc.dram_tensor("xT16", [KF16 * KP, NT], FP16, kind="ExternalInput")
    # only the last 2 k-chunks of fp8 x come from DRAM (rest is DVE-cast)
    xT8t = nc.dram_tensor("xT8t", [(KCH - KF16) * KP, NT], FP8,
                          kind="ExternalInput")
    wp16 = nc.dram_tensor("wp16", [KF16 * KP, D_HID], FP16, kind="ExternalInput")
    wp8 = nc.dram_tensor("wp8", [(KCH - KF16) * KP, D_HID], FP8, kind="ExternalInput")
    wq8 = nc.dram_tensor("wq8", [D_IN, D_HID], FP8, kind="ExternalInput")
    # per-token epilogue scalars, partition-major: sC[p, m, :] = (u', v') of
    # token m*128 + p (PCA-basis scores, pre-divided by the 4096 data scale)
    sC = nc.dram_tensor("sC", [128, NT // 128, 2], F32, kind="ExternalInput")
    out = nc.dram_tensor("out", [NT, D_HID], O_DT, kind="ExternalOutput")

    MULT = mybir.AluOpType.mult
    ADD = mybir.AluOpType.add

    with tile.TileContext(nc) as tc:
        with (
            tc.tile_pool(name="cst", bufs=1) as cst,
            tc.tile_pool(name="wm", bufs=1) as wm,
            tc.tile_pool(name="wp", bufs=3) as wp,
            tc.tile_pool(name="xp", bufs=1) as xp,
            tc.tile_pool(name="ep", bufs=6) as ep,
            # o-tiles get deep rotation: store descriptors can lag ~16
            # groups behind (ring-credit-serialized queues) without the
            # DVE WAR-stalling on a not-yet-drained store
            tc.tile_pool(name="op", bufs=16) as op,
            tc.tile_pool(name="ps", bufs=4, space=bass.MemorySpace.PSUM) as ps,
        ):
            # HAM warm-up: a chain of matmuls on a DVE-zeroed tile with no
            # DMA dependencies.  The target PSUM tile is one rotation slot
            # of the pa tag, never read, fully overwritten later.
            wz = wm.tile([KP, NB], FP16, tag="wz")
            nc.vector.memset(wz[:], 0.0)
            pwt = ps.tile([128, NB], F32, tag="pa")
            for _ in range(NWARM):
                nc.tensor.matmul(pwt[:], wz[:, 0:128], wz[:], start=True, stop=True)

            def _dma3(t, dram, eng, col0, k0, kn, toff=0):
                eng.dma_start(
                    t[:, toff + k0:toff + k0 + kn, :],
                    dram[k0 * KP:(k0 + kn) * KP, col0:col0 + t.shape[2]].rearrange(
                        "(j p) t -> p j t", p=KP),
                )

            x_tiles = {}

            def _alloc_x(q):
                t16 = xp.tile([KP, KF16, TQ], FP16, tag=f"xs16_{q}", name=f"x16_{q}")
                t8 = xp.tile([KP, KCH, TQ], FP8, tag=f"xs8_{q}", name=f"x8_{q}")
                x_tiles[q] = (t16, t8)
                return t16, t8

            # x stays SBUF-resident for the whole kernel: each slice's fp16
            # is loaded exactly once; its fp8 copy is DVE-cast on device.
            def load_x(q):
                t16, t8 = _alloc_x(q)
                e0, e1 = (nc.sync, nc.scalar) if q % 2 == 0 else (nc.scalar, nc.sync)
                _dma3(t16, xT16, e0, q * TQ, 0, KF16 // 2)
                _dma3(t16, xT16, e1, q * TQ, KF16 // 2, KF16 - KF16 // 2)
                _dma3(t8, xT8t, e0, q * TQ, 0, KCH - KF16, toff=KF16)
                return x_tiles[q]

            def cast_x(q):
                t16, t8 = x_tiles[q]
                nc.vector.tensor_copy(t8[:, 0:KF16, :], t16[:, :, :])

            # W tiles per block: wp16 [128, KF16, NB] fp16, wp8 [128, 2, NB]
            # fp8 (P-term k-chunks 14..15), wq8 [128, KCH, NB] fp8
            def load_w(nb):
                c0 = nb * NB
                t16 = wp.tile([KP, KF16, NB], FP16, tag="w16", name=f"w16_{nb}")
                t8p = wp.tile([KP, KCH - KF16, NB], FP8, tag="wp8", name=f"wp8_{nb}")
                t8q = wp.tile([KP, KCH, NB], FP8, tag="wq8", name=f"wq8_{nb}")
                h = KF16 // 2
                _dma3(t16, wp16, nc.sync, c0, 0, h)
                _dma3(t16, wp16, nc.scalar, c0, h, KF16 - h)
                _dma3(t8q, wq8, nc.sync, c0, 0, KCH)
                _dma3(t8p, wp8, nc.scalar, c0, 0, KCH - KF16)
                return t16, t8p, t8q

            # cold fill: x16(q0, q1) + W(block0) in progressive k-levels,
            # first pieces smallest; per-queue issue kept coarse (>=128KB
            # per start where possible).  fp8 x for q0/q1 is cast piecewise
            # as its fp16 levels land.
            def cold_fill():
                tx0 = _alloc_x(0)
                tx1 = _alloc_x(1)
                t16 = wp.tile([KP, KF16, NB], FP16, tag="w16", name="w16_c")
                t8p = wp.tile([KP, KCH - KF16, NB], FP8, tag="wp8", name="wp8_c")
                t8q = wp.tile([KP, KCH, NB], FP8, tag="wq8", name="wq8_c")

                def xlv(k0, kn):
                    # one x16 level for both slices + the DVE cast of its
                    # fp8 copy (DVE is otherwise idle during the cold fill)
                    _dma3(tx0[0], xT16, nc.scalar, 0, k0, kn)
                    _dma3(tx1[0], xT16, nc.scalar, TQ, k0, kn)
                    nc.vector.tensor_copy(tx0[1][:, k0:k0 + kn, :],
                                          tx0[0][:, k0:k0 + kn, :])
                    nc.vector.tensor_copy(tx1[1][:, k0:k0 + kn, :],
                                          tx1[0][:, k0:k0 + kn, :])

                # per-queue emission order is deadline-driven (k-pair p of
                # the super-group is consumed at ~8.2 + 2.63*p us):
                # sync   = W-for-the-supergroup (wp16/wq8 low + wp8 + x8 tails)
                # scalar = x16 both slices + wq8 high half + sC
                _dma3(t16, wp16, nc.sync, 0, 0, 2)
                xlv(0, 2)
                _dma3(t8q, wq8, nc.sync, 0, 0, 2)
                nc.scalar.dma_start(sC_sb[:], sC[:])
                _dma3(t16, wp16, nc.sync, 0, 2, 2)
                xlv(2, 2)
                _dma3(t8q, wq8, nc.sync, 0, 2, 2)
                _dma3(tx0[1], xT8t, nc.sync, 0, 0, KCH - KF16, toff=KF16)
                _dma3(tx1[1], xT8t, nc.sync, TQ, 0, KCH - KF16, toff=KF16)
                _dma3(t8q, wq8, nc.scalar, 0, 8, 4)
                _dma3(t8q, wq8, nc.sync, 0, 4, 4)
                xlv(4, 4)
                _dma3(t16, wp16, nc.sync, 0, 4, 4)
                _dma3(t8q, wq8, nc.scalar, 0, 12, 4)
                _dma3(t16, wp16, nc.sync, 0, 8, 2)
                xlv(8, 2)
                _dma3(t16, wp16, nc.sync, 0, 10, 4)
                xlv(10, 4)
                _dma3(t8p, wp8, nc.sync, 0, 0, KCH - KF16)
                return (t16, t8p, t8q)

            def epilogue(pa, pb, mg, nb, last=False):
                nb_sl = bass.ts(nb, NB)
                s0 = sC_sb[:, mg, 0:1]
                s1 = sC_sb[:, mg, 1:2]
                # out = u'*pa + v'*pb on DVE (each op reads one PSUM input)
                t1 = ep.tile([128, NB], O_DT, tag="t1")
                nc.vector.tensor_scalar_mul(t1[:], pa[:], s0)
                o = op.tile([128, NB], O_DT, tag="o")
                nc.vector.scalar_tensor_tensor(
                    o[:], pb[:], s1, t1[:], op0=MULT, op1=ADD
                )
                m_sl = bass.ts(mg, 128)
                if last:
                    # split the final store across both queues to shorten
                    # the kernel tail
                    h = NB // 2
                    c0 = nb * NB
                    nc.sync.dma_start(out[m_sl, c0:c0 + h], o[:, 0:h])
                    nc.scalar.dma_start(out[m_sl, c0 + h:c0 + NB], o[:, h:NB])
                else:
                    eng = nc.sync if mg % 2 == 0 else nc.scalar
                    eng.dma_start(out[m_sl, nb_sl], o[:])

            # one (m-tile, nb) group, j-interleaved: 2 fp16 P-MMs then 1 DR
            # Q-MM per k-pair; tail = shared-lhsT DR pair (P fp8 k14..15 +
            # Q k-pair 7).
            def mm_j(pa, pb, xt, w_t, j, mi, csl=slice(0, NB)):
                (t16x, t8x), (w16, w8p, w8q) = xt, w_t
                m_sl = bass.ts(mi, 128)
                if j < KJ - 1:
                    for k in (2 * j, 2 * j + 1):
                        nc.tensor.matmul(pa[:, :], t16x[:, k, m_sl],
                                         w16[:, k, csl],
                                         start=(k == 0), stop=False)
                    nc.tensor.matmul(pb[:, :], t8x[:, 2 * j:2 * j + 2, m_sl],
                                     w8q[:, 2 * j:2 * j + 2, csl],
                                     start=(j == 0), stop=False, perf_mode=DR)
                else:
                    nc.tensor.matmul(pa[:, :], t8x[:, KF16:KCH, m_sl],
                                     w8p[:, :, csl],
                                     start=False, stop=True, perf_mode=DR)
                    nc.tensor.matmul(pb[:, :], t8x[:, KF16:KCH, m_sl],
                                     w8q[:, KF16:KCH, csl],
                                     start=False, stop=True, perf_mode=DR)

            def group(q, mi, nb, w_t, last=False):
                x_t = x_tiles[q]
                mg = q * MPQ + mi
                if last:
                    # final group: run as two column-halves so the first
                    # half's epilogue + store overlap the second half's
                    # matmuls (shorter kernel tail)
                    h = NB // 2
                    for hf in range(2):
                        pa = ps.tile([128, h], F32, tag="pa", name=f"pa_l{hf}")
                        pb = ps.tile([128, h], F32, tag="pb", name=f"pb_l{hf}")
                        c_sl = slice(hf * h, (hf + 1) * h)
                        for j in range(KJ):
                            mm_j(pa, pb, x_t, w_t, j, mi, c_sl)
                        s0 = sC_sb[:, mg, 0:1]
                        s1 = sC_sb[:, mg, 1:2]
                        t1 = ep.tile([128, h], O_DT, tag="t1", name=f"t1_l{hf}")
                        nc.vector.tensor_scalar_mul(t1[:], pa[:], s0)
                        o = op.tile([128, h], O_DT, tag="o", name=f"o_l{hf}")
                        nc.vector.scalar_tensor_tensor(
                            o[:], pb[:], s1, t1[:], op0=MULT, op1=ADD)
                        c0 = nb * NB + hf * h
                        eng = nc.sync if hf == 0 else nc.scalar
                        eng.dma_start(out[bass.ts(mg, 128), c0:c0 + h], o[:])
                    return
                pa = ps.tile([128, NB], F32, tag="pa")
                pb = ps.tile([128, NB], F32, tag="pb")
                for j in range(KJ):
                    mm_j(pa, pb, x_t, w_t, j, mi)
                epilogue(pa, pb, mg, nb)

            # ---- phase 1: slices q0, q1 across ALL four blocks ----------
            sC_sb = cst.tile([128, NT // 128, 2], F32)
            w_t = {0: cold_fill()}
            w_t[1] = load_w(1)
            w_t[2] = load_w(2)
            load_x(2)
            load_x(3)

            # block 0 over q0/q1: ONE k-outer super-group over 4 m-groups
            # and all 8 PSUM banks, in strict k arrival order.  The Q-term
            # DRs LAG the fp16 stream by one k-pair (relaxes every cold
            # wq8/cast deadline by 2.6us); each group then finishes its own
            # tail (Q6, P-DR, Q7) followed immediately by its epilogue, so
            # PSUM banks free up while later groups' tails still run.
            pas = [ps.tile([128, NB], F32, tag="pa", name=f"pa_c{g}")
                   for g in range(4)]
            pbs = [ps.tile([128, NB], F32, tag="pb", name=f"pb_c{g}")
                   for g in range(4)]
            for j in range(KJ - 1):
                for k in (2 * j, 2 * j + 1):
                    for g in range(4):
                        q, mi = divmod(g, MPQ)
                        nc.tensor.matmul(
                            pas[g][:, :],
                            x_tiles[q][0][:, k, bass.ts(mi, 128)],
                            w_t[0][0][:, k, :],
                            start=(k == 0), stop=False)
                if j < KJ - 2:
                    for g in range(4):
                        q, mi = divmod(g, MPQ)
                        nc.tensor.matmul(
                            pbs[g][:, :],
                            x_tiles[q][1][:, 2 * j:2 * j + 2, bass.ts(mi, 128)],
                            w_t[0][2][:, 2 * j:2 * j + 2, :],
                            start=(j == 0), stop=False, perf_mode=DR)
            for g in range(4):
                q, mi = divmod(g, MPQ)
                m_sl = bass.ts(mi, 128)
                nc.tensor.matmul(
                    pbs[g][:, :], x_tiles[q][1][:, 2 * (KJ - 2):2 * KJ - 2, m_sl],
                    w_t[0][2][:, 2 * (KJ - 2):2 * KJ - 2, :],
                    start=False, stop=False, perf_mode=DR)
                nc.tensor.matmul(
                    pas[g][:, :], x_tiles[q][1][:, KF16:KCH, m_sl],
                    w_t[0][1][:, :, :], start=False, stop=True, perf_mode=DR)
                nc.tensor.matmul(
                    pbs[g][:, :], x_tiles[q][1][:, KF16:KCH, m_sl],
                    w_t[0][2][:, KF16:KCH, :], start=False, stop=True,
                    perf_mode=DR)
                epilogue(pas[g], pbs[g], g, 0)

            load_x(2)
            load_x(3)

            # blocks 1..3 over q0/q1.  W3 and the remaining x-slice loads
            # are deferred past nb1: the HWDGE rings hold only ~64
            # descriptors, and a deep prefetch backlog was observed to
            # strand epilogue-store descriptors (and their o-tile WARs)
            # behind 30us of queued transfers.  x-casts for phase 2 are
            # slotted between blocks (x16 has long arrived -> no DVE FIFO
            # stall).
            # casts are spread one-per-group across nb2/nb3 groups (a 2us
            # cast on the strict-FIFO DVE right at a block boundary was
            # observed to strand the epilogues that recycle PSUM banks)
            casts = {(2, 0): 2, (2, 1): 3, (2, 2): 4, (2, 3): 5,
                     (3, 0): 6, (3, 1): 7}
            for nb in (1, 2, 3):
                for q in (0, 1):
                    for mi in range(MPQ):
                        group(q, mi, nb, w_t[nb])
                        cq = casts.get((nb, q * MPQ + mi))
                        if cq is not None:
                            cast_x(cq)
                if nb == 1:
                    w_t[3] = load_w(3)
                    for q2 in range(4, NQ):
                        load_x(q2)

            # ---- phase 2: blocks [3,2,1,0] over slices q2..q7 -----------
            # W3 is still resident; each block prefetches the next block's
            # W at its start (before its epilogue stores hit the queues).
            order = (3, 2, 1, 0)
            w_cur = w_t[3]
            for i, nb in enumerate(order):
                w_next = load_w(order[i + 1]) if i + 1 < len(order) else None
                for q in range(2, NQ):
                    for mi in range(MPQ):
                        last = (i == len(order) - 1 and q == NQ - 1
                                and mi == MPQ - 1)
                        group(q, mi, nb, w_cur, last=last)
                w_cur = w_next

    nc.compile()
    return nc


def _host_gating(x, W_gate, b_gate):
    logits = x @ W_gate + b_gate                       # [N, 8] fp32
    m = logits.max(axis=1, keepdims=True)
    e = np.exp(logits - m)
    gates = e / e.sum(axis=1, keepdims=True)
    idx0 = np.argsort(-gates[0], kind="stable")[:2]    # token-0 top-2 experts
    scores = -np.sort(-gates, axis=1)[:, :2]           # per-token top-2 values
    return idx0, np.ascontiguousarray(scores)


def kernel(x, W_experts, b_experts, W_gate, b_gate):
    global LAST_RESULT
    x = np.ascontiguousarray(np.asarray(x, dtype=np.float32))
    W_experts = np.asarray(W_experts, dtype=np.float32)
    b_experts = np.asarray(b_experts, dtype=np.float32)
    W_gate = np.asarray(W_gate, dtype=np.float32)
    b_gate = np.asarray(b_gate, dtype=np.float32)

    idx0, scores = _host_gating(x, W_gate, b_gate)

    # PCA (2nd moment) of the (s0, s1) cloud -> major/minor basis
    Sig = scores.T.astype(np.float64) @ scores.astype(np.float64) / N
    lam, V = np.linalg.eigh(Sig)                       # ascending
    Mrot = np.stack([V[:, 1], V[:, 0]])                # rows: major, minor
    uv = (scores.astype(np.float64) @ np.linalg.inv(Mrot)).astype(np.float32)

    WA = W_experts[idx0[0]]
    WB = W_experts[idx0[1]]
    WP = (Mrot[0, 0] * WA + Mrot[0, 1] * WB).astype(np.float32)
    WQ = (Mrot[1, 0] * WA + Mrot[1, 1] * WB).astype(np.float32)

    f8 = mybir.dt.np(FP8)
    kf = KF16 * KP
    wp16_np = (WP[:kf] * SW).astype(np.float16)
    wp8_np = np.clip(WP[kf:] * SW, -240, 240).astype(f8)
    wq8_np = np.clip(WQ * SW, -240, 240).astype(f8)

    xT16_full = np.ascontiguousarray((x.T[:kf] * SX).astype(np.float16))
    xT8t_full = np.ascontiguousarray(
        np.clip(x.T[kf:] * SX, -240, 240).astype(f8))

    uv_sc = uv / (SX * SW)                             # epilogue scalars

    nc = _build()
    in_maps = []
    for c in range(N_CORES):
        sl = slice(c * NT, (c + 1) * NT)
        in_maps.append(
            {
                "xT16": np.ascontiguousarray(xT16_full[:, sl]),
                "xT8t": np.ascontiguousarray(xT8t_full[:, sl]),
                "wp16": wp16_np,
                "wp8": wp8_np,
                "wq8": wq8_np,
                "sC": np.ascontiguousarray(
                    uv_sc[sl].reshape(NT // 128, 128, 2).transpose(1, 0, 2)
                ),
            }
        )

    res = run_bass_kernel_spmd(nc, in_maps, list(range(N_CORES)))
    LAST_RESULT = res
    out = np.concatenate(
        [r["out"] for r in res.results], axis=0
    ).astype(np.float32)
    # bias term s0*bA + s1*bB is a rank-2 correction, added here in fp32
    out += scores @ b_experts[idx0]
    return out
